# revision 6
# baseline (speedup 1.0000x reference)
"""EMD (Sinkhorn) loss kernel for Trainium2, 8 NeuronCores.

Reference computes, for each (q, p) pair of a 128x128 grid, a 100-iteration
entropic-regularized Sinkhorn transport solve on a 32x32 cost matrix, then
logits[q, p] = sum(flow * sim) * (T / 32).

Device formulation (exp-domain Sinkhorn, validated to ~2e-6 of the jax
log-domain reference):
    K  = exp((sim - 1) / eps)
    w0 = 1 / b
    repeat N_ITERS:  r = (K*b) @ w ;  s = (K*a)^T @ (1/r) ;  w = 1/s
    logits = sum_ij (a_i / r_i) * K_ij * sim_ij * (b_j * w_j)

Sharding: data-parallel over q (16 q per core, 2048 independent 32x32
problems per core).

Per-core device layout ("fused reduce+transpose" scheme):
  pair id g = q_local*128 + p,  alpha = g >> 9 (2b), beta = g & 511 (9b)
  A-side tiles: partition = (alpha, j), free = (beta, i)
  B-side tiles: partition = (beta&3, i), free = (F, j), F = (beta>>2)*4 + alpha
  The j-reduction of an A-side product tile is done on the tensor engine with
  the product tile as the *stationary* operand and a [128, 4] block-diagonal
  ones matrix as the *moving* operand: out[(bl,i), 4m+alpha] = sum_j tmp.
  That lands r directly in the B-side layout (and vice versa), so the
  iteration needs no separate transposes and the layout is self-consistent.
"""

import math
import numpy as np

EPS = 0.05
N_ITERS = 100
TEMP = 12.5
Q, P, N1, N2 = 128, 128, 32, 32
N_CORES = 8
QL = Q // N_CORES          # 16 queries per core
NPAIR = QL * P             # 2048 pairs per core
NB = 512                   # beta-space size (NPAIR // 4)
FREE = NB * 32             # 16384 free elements per partition for K tiles
N_STREAMS = 2              # independent pair-halves for pipelining
NB_S = NB // N_STREAMS     # 256 betas per stream
CHUNK_B = 64               # betas per DVE chunk (=> 2048 free elems)
CHUNK = CHUNK_B * 32


def _marginals(lengths, n):
    mask = (np.arange(n)[None, :] < np.asarray(lengths)[:, None]).astype(np.float32)
    w = mask + np.float32(1e-5)
    return w / w.sum(-1, keepdims=True, dtype=np.float32)


def pack_core(sim_c, a_c, b):
    """sim_c: [QL, P, 32, 32] f32, a_c: [QL, 32], b: [P, 32].
    Returns dict of packed per-core device inputs (all f32)."""
    K = np.exp((sim_c.astype(np.float64) - 1.0) / EPS).astype(np.float32)
    K4 = K.reshape(4, NB, N1, N2)                    # [alpha, beta, i, j]
    sim4 = sim_c.reshape(4, NB, N1, N2)
    gp = np.arange(NPAIR)
    a_pair = a_c[gp // P]                            # [2048, 32] over i
    b_pair = b[gp % P]                               # [2048, 32] over j
    a4 = a_pair.reshape(4, NB, N1, 1)
    b4 = b_pair.reshape(4, NB, 1, N2)

    # ka[(alpha, j), (beta, i)] = b_j * K
    ka = np.ascontiguousarray(
        (K4 * b4).transpose(0, 3, 1, 2)              # [alpha, j, beta, i]
    ).reshape(128, FREE)
    # kb[(bl, i), (m, alpha, j)] = a_i * K   (beta = 4m + bl)
    KA5 = (K4 * a4).reshape(4, NB // 4, 4, N1, N2)   # [alpha, m, bl, i, j]
    kb = np.ascontiguousarray(
        KA5.transpose(2, 3, 1, 0, 4)                 # [bl, i, m, alpha, j]
    ).reshape(128, FREE)
    # mb: same layout as kb, value a_i * b_j * sim * K * (TEMP/N1)
    MB5 = (K4 * a4 * b4 * sim4 * np.float32(TEMP / N1)).reshape(4, NB // 4, 4, N1, N2)
    mb = np.ascontiguousarray(MB5.transpose(2, 3, 1, 0, 4)).reshape(128, FREE)
    # w0[(alpha, j), beta] = 1 / b_j
    w0 = np.ascontiguousarray(
        (1.0 / b_pair.astype(np.float64)).astype(np.float32)
        .reshape(4, NB, N2).transpose(0, 2, 1)
    ).reshape(128, NB)
    ones4 = np.kron(np.eye(4, dtype=np.float32), np.ones((32, 1), np.float32))
    return {"ka": ka, "kb": kb, "mb": mb, "w0": w0, "ones4": ones4}


def unpack_logits(L):
    """L: [128, 16] device output -> [QL, P] logits."""
    # L[bpos, 4c + alpha] = logits[pair(alpha, 128c + bpos)]
    return np.ascontiguousarray(
        L.reshape(128, 4, 4).transpose(2, 1, 0)      # [alpha, c, bpos]
    ).reshape(QL, P)


def device_sim_numpy(packed, n_iters=N_ITERS):
    """Numpy emulation of the device program, layout-for-layout."""
    ka, kb, mb, w0 = (packed[k] for k in ("ka", "kb", "mb", "w0"))
    W = w0.copy()                                    # [(alpha, j), beta]
    kav = ka.reshape(128, NB, N1)                    # [(a,j), beta, i]
    kbv = kb.reshape(128, NB, N2)                    # [(bl,i), F, j]
    mbv = mb.reshape(128, NB, N2)
    for t in range(n_iters):
        tmp1 = kav * W[:, :, None]                   # [(a,j), beta, i]
        # r-reduce: R[(bl,i), 4m+alpha] = sum_j tmp1[(alpha,j), (4m+bl, i)]
        t1 = tmp1.reshape(4, 32, NB // 4, 4, 32)      # [alpha, j, m, bl, i]
        R = t1.sum(1).transpose(2, 3, 1, 0).reshape(128, NB)   # [bl,i,m,alpha]
        RI = 1.0 / R
        tmp2 = kbv * RI[:, :, None]                  # [(bl,i), F, j]
        t2 = tmp2.reshape(4, 32, NB // 4, 4, 32)      # [bl, i, m2, gl, j]
        S = t2.sum(1).transpose(2, 3, 1, 0).reshape(128, NB)   # [gl,j,m2,bl]
        W = 1.0 / S
    tmpf = mbv * RI[:, :, None]
    tf = tmpf.reshape(4, 32, NB // 4, 4, 32)
    G = tf.sum(1).transpose(2, 3, 1, 0).reshape(128, NB)
    P2 = G * W                                       # [(alpha, j), beta]
    # L[bpos, 4c+alpha] = sum_j P2[(alpha, j), 128c + bpos]
    p2 = P2.reshape(4, 32, 4, 128)                   # [alpha, j, c, bpos]
    L = p2.sum(1).transpose(2, 1, 0).reshape(128, 16)
    return L


def build_program(n_iters=N_ITERS):
    from contextlib import ExitStack
    from concourse import bacc, tile, mybir

    nc = bacc.Bacc("TRN2", target_bir_lowering=False, debug=False,
                   enable_asserts=False, num_devices=N_CORES)
    f32 = mybir.dt.float32
    ka_d = nc.dram_tensor("ka", [128, FREE], f32, kind="ExternalInput")
    kb_d = nc.dram_tensor("kb", [128, FREE], f32, kind="ExternalInput")
    mb_d = nc.dram_tensor("mb", [128, FREE], f32, kind="ExternalInput")
    w0_d = nc.dram_tensor("w0", [128, NB], f32, kind="ExternalInput")
    on_d = nc.dram_tensor("ones4", [128, 4], f32, kind="ExternalInput")
    out_d = nc.dram_tensor("out", [128, 16], f32, kind="ExternalOutput")

    with tile.TileContext(nc) as tc:
        _emd_body(tc, n_iters, ka_d, kb_d, mb_d, w0_d, on_d, out_d)
    nc.compile()
    return nc


def _emd_body(tc, n_iters, ka_d, kb_d, mb_d, w0_d, on_d, out_d):
    from concourse import mybir
    nc = tc.nc
    f32 = mybir.dt.float32
    AF = mybir.ActivationFunctionType

    import contextlib
    ctx = contextlib.ExitStack()
    singles = ctx.enter_context(tc.tile_pool(name="singles", bufs=1))
    tmpp = ctx.enter_context(tc.tile_pool(name="tmpp", bufs=3))
    mbp = ctx.enter_context(tc.tile_pool(name="mbp", bufs=2))
    potp = ctx.enter_context(tc.tile_pool(name="potp", bufs=2))
    psr = ctx.enter_context(tc.tile_pool(name="psr", bufs=1, space="PSUM"))
    pss = ctx.enter_context(tc.tile_pool(name="pss", bufs=1, space="PSUM"))
    psl = ctx.enter_context(tc.tile_pool(name="psl", bufs=1, space="PSUM"))

    ka = singles.tile_from(ka_d.ap())        # [128, FREE]
    kb = singles.tile_from(kb_d.ap())
    w0 = singles.tile_from(w0_d.ap())        # [128, NB]
    ones4 = singles.tile_from(on_d.ap())     # [128, 4]

    nchunk_s = (NB_S * 32) // CHUNK          # chunks per stream per half-iter
    mm_per_chunk = CHUNK // 128              # stationary slices per chunk

    # per-stream potential tiles (SBUF); W[s] layout [(alpha,j), beta_s]
    W = [None] * N_STREAMS
    RI = [None] * N_STREAMS
    LN = [None] * N_STREAMS

    def half_iter(t, side, s):
        """side 0: r-step (ka, reads W -> writes RI); side 1: s-step."""
        if side == 0:
            ksrc, pin, ppool = ka, (w0 if t == 0 else W[s]), psr
        else:
            ksrc, pin, ppool = kb, RI[s], pss
        base = s * NB_S                       # beta/F offset of this stream
        pr = ppool.tile([128, NB_S], f32, name=f"ps_{side}_{s}", tag=f"ps{side}{s}")
        for c in range(nchunk_s):
            off = (base + c * CHUNK_B) * 32
            tmp = tmpp.tile([128, CHUNK], f32, name="tmp", tag="tmp")
            # in1: pin[:, beta-slice] broadcast over innermost 32 (stride 0)
            bsl = pin[:, base + c * CHUNK_B: base + (c + 1) * CHUNK_B]
            nc.vector.tensor_mul(
                out=tmp[:].rearrange("p (b i) -> p b i", i=32),
                in0=ksrc[:, off: off + CHUNK].rearrange("p (b i) -> p b i", i=32),
                in1=bsl.broadcast_to([128, CHUNK_B, 32]),
            )
            for k in range(mm_per_chunk):
                m = c * mm_per_chunk + k
                nc.tensor.matmul(
                    out=pr[:, 4 * m: 4 * m + 4],
                    lhsT=tmp[:, 128 * k: 128 * (k + 1)],
                    rhs=ones4[:],
                    start=True, stop=True,
                )
        # reciprocal via exp(-ln(x)) on the scalar engine
        lnb = potp.tile([128, NB_S], f32, name=f"ln_{side}_{s}", tag=f"ln{s}")
        nc.scalar.activation(out=lnb[:], in_=pr[:], func=AF.Ln)
        if side == 0:
            dst = potp.tile([128, NB], f32, name=f"ri_{s}", tag=f"ri{s}")
            RI[s] = dst
            dsl = dst[:, base: base + NB_S]
        else:
            dst = potp.tile([128, NB], f32, name=f"w_{s}", tag=f"w{s}")
            W[s] = dst
            dsl = dst[:, base: base + NB_S]
        nc.scalar.activation(out=dsl, in_=lnb[:], func=AF.Exp, scale=-1.0)
        return pr

    for t in range(n_iters):
        for s in range(N_STREAMS):
            half_iter(t, 0, s)
        for s in range(N_STREAMS):
            half_iter(t, 1, s)

    # ---- final pass ----
    p2 = singles.tile([128, NB], f32, name="p2")
    for s in range(N_STREAMS):
        base = s * NB_S
        gps = pss.tile([128, NB_S], f32, name=f"gps_{s}", tag=f"ps1{s}")
        for c in range(nchunk_s):
            off = (base + c * CHUNK_B) * 32
            mbc = mbp.tile([128, CHUNK], f32, name="mbc", tag="mbc")
            nc.sync.dma_start(mbc[:], mb_d.ap()[:, off: off + CHUNK])
            tmp = tmpp.tile([128, CHUNK], f32, name="tmp", tag="tmp")
            bsl = RI[s][:, base + c * CHUNK_B: base + (c + 1) * CHUNK_B]
            nc.vector.tensor_mul(
                out=tmp[:].rearrange("p (b i) -> p b i", i=32),
                in0=mbc[:].rearrange("p (b i) -> p b i", i=32),
                in1=bsl.broadcast_to([128, CHUNK_B, 32]),
            )
            for k in range(mm_per_chunk):
                m = c * mm_per_chunk + k
                nc.tensor.matmul(
                    out=gps[:, 4 * m: 4 * m + 4],
                    lhsT=tmp[:, 128 * k: 128 * (k + 1)],
                    rhs=ones4[:], start=True, stop=True,
                )
        nc.vector.tensor_mul(
            out=p2[:, base: base + NB_S], in0=gps[:], in1=W[s][:, base: base + NB_S]
        )
    lps = psl.tile([128, 16], f32, name="lps")
    for c in range(4):
        nc.tensor.matmul(
            out=lps[:, 4 * c: 4 * c + 4],
            lhsT=p2[:, 128 * c: 128 * (c + 1)],
            rhs=ones4[:], start=True, stop=True,
        )
    outsb = singles.tile([128, 16], f32, name="outsb")
    nc.vector.tensor_copy(outsb[:], lps[:])
    nc.sync.dma_start(out_d.ap(), outsb[:])
    ctx.close()


_NC_CACHE = {}


def _get_program(n_iters=N_ITERS):
    if n_iters not in _NC_CACHE:
        _NC_CACHE[n_iters] = build_program(n_iters)
    return _NC_CACHE[n_iters]


def kernel(similarity_map, im_set, s_seq, im_len, s_len):
    sim = np.ascontiguousarray(np.asarray(similarity_map, dtype=np.float32))
    a = _marginals(np.asarray(im_len), N1)           # [Q, 32]
    b = _marginals(np.asarray(s_len), N2)            # [P, 32]

    nc = _get_program(N_ITERS)
    in_maps = []
    for c in range(N_CORES):
        sim_c = sim[c * QL:(c + 1) * QL]
        a_c = a[c * QL:(c + 1) * QL]
        in_maps.append(pack_core(sim_c, a_c, b))

    from concourse.bass_utils import run_bass_kernel_spmd
    res = run_bass_kernel_spmd(nc, in_maps, core_ids=list(range(N_CORES)))
    out = np.concatenate(
        [unpack_logits(res.results[c]["out"]) for c in range(N_CORES)], axis=0
    )
    return out.astype(np.float32)


# revision 9
# speedup vs baseline: 2.1784x; 2.1784x over previous
"""EMD (Sinkhorn) loss kernel for Trainium2, 8 NeuronCores.

Reference: for each (q, p) pair of a 128x128 grid, run a 100-iteration
entropic Sinkhorn solve on a 32x32 cost matrix; logits[q,p] = sum(flow*sim)
* (12.5/32).

Exp-domain formulation (matches the jax log-domain reference to ~2e-6):
    K = exp((sim-1)/eps);  v0 = 1
    repeat: r_i = sum_j K_ij v_j ; u = a/r ; s_j = sum_i K_ij u_i ; v = b/s
    logits = sum_ij u_i K_ij v_j sim_ij * (T/32)

Sharding: data-parallel over q (16 q / core -> 2048 independent 32x32
problems per core).

This environment executes ~1 instruction per ~35us regardless of size
(measured), so the kernel minimizes INSTRUCTION COUNT: one big SBUF tile
holds all 2048 problems ([128 partitions, 16 pairs x 32 x 32]); each
Sinkhorn half-step is one full-tile tensor_tensor multiply + one grouped
tensor_reduce; reciprocals on [128, 512] potentials. 8 instructions per
iteration.
"""

import numpy as np

EPS = 0.05
N_ITERS = 100
TEMP = 12.5
Q, P, N1, N2 = 128, 128, 32, 32
N_CORES = 8
QL = Q // N_CORES          # 16 queries per core
NPAIR = QL * P             # 2048 pairs per core
PL = NPAIR // 128          # 16 pairs per partition
FREE = PL * N1 * N2        # 16384
POT = PL * 32              # 512 potential values per partition


def _marginals(lengths, n):
    mask = (np.arange(n)[None, :] < np.asarray(lengths)[:, None]).astype(np.float32)
    w = mask + np.float32(1e-5)
    return w / w.sum(-1, keepdims=True, dtype=np.float32)


def pack_core(sim_c, a_c, b):
    """sim_c: [QL, P, 32, 32] f32, a_c: [QL, 32], b: [P, 32] -> device inputs."""
    K = np.exp((sim_c.astype(np.float64) - 1.0) / EPS).astype(np.float32)
    k = K.reshape(128, FREE)                       # pair f = q*128+p -> (f//16, f%16)
    m = (K * sim_c * np.float32(TEMP / N1)).reshape(128, FREE)
    f = np.arange(NPAIR)
    apre = a_c[f >> 7].reshape(128, POT)           # [p, (pl, i)]
    bpre = b[f & 127].reshape(128, POT)            # [p, (pl, j)]
    return {"k": k, "m": m, "apre": apre, "bpre": bpre}


def unpack_logits(L):
    """L: [128, 16] -> [QL, P]."""
    return L.reshape(QL, P).copy()


def device_sim_numpy(packed, n_iters=N_ITERS):
    k4 = packed["k"].reshape(128, PL, N1, N2)
    m4 = packed["m"].reshape(128, PL, N1, N2)
    a = packed["apre"].reshape(128, PL, N1)
    b = packed["bpre"].reshape(128, PL, N2)
    for t in range(n_iters):
        if t == 0:
            r = k4.sum(-1)                          # [128, PL, 32]
        else:
            v = b * w
            r = (k4 * v[:, :, None, :]).sum(-1)
        u = a / r
        s = (k4 * u[:, :, :, None]).sum(-2)         # [128, PL, 32]
        w = 1.0 / s
    v = b * w
    t1 = m4 * v[:, :, None, :]
    t2 = t1 * u[:, :, :, None]
    return t2.sum((-1, -2)).reshape(128, PL)


def build_program(n_iters=N_ITERS):
    from contextlib import ExitStack
    from concourse import bacc, tile, mybir

    nc = bacc.Bacc("TRN2", target_bir_lowering=False, debug=False,
                   enable_asserts=False, num_devices=N_CORES)
    f32 = mybir.dt.float32
    k_d = nc.dram_tensor("k", [128, FREE], f32, kind="ExternalInput")
    m_d = nc.dram_tensor("m", [128, FREE], f32, kind="ExternalInput")
    a_d = nc.dram_tensor("apre", [128, POT], f32, kind="ExternalInput")
    b_d = nc.dram_tensor("bpre", [128, POT], f32, kind="ExternalInput")
    out_d = nc.dram_tensor("out", [128, PL], f32, kind="ExternalOutput")

    with tile.TileContext(nc) as tc:
        _emd_body(tc, n_iters, k_d, m_d, a_d, b_d, out_d)
    nc.compile()
    return nc


def _emd_body(tc, n_iters, k_d, m_d, a_d, b_d, out_d):
    from contextlib import ExitStack
    from concourse import mybir
    import concourse.bass as bass
    nc = tc.nc
    f32 = mybir.dt.float32
    ADD = mybir.AluOpType.add
    X = mybir.AxisListType.X
    XY = mybir.AxisListType.XY

    ctx = ExitStack()
    sp = ctx.enter_context(tc.tile_pool(name="sp", bufs=1))

    k = sp.tile_from(k_d.ap())                      # [128, FREE]
    apre = sp.tile_from(a_d.ap())                   # [128, POT]
    bpre = sp.tile_from(b_d.ap())
    tmp = sp.tile([128, FREE], f32, name="tmp")
    v = sp.tile([128, POT], f32, name="v")
    r = sp.tile([128, POT], f32, name="r")
    ri = sp.tile([128, POT], f32, name="ri")
    u = sp.tile([128, POT], f32, name="u")
    s = sp.tile([128, POT], f32, name="s")
    w = sp.tile([128, POT], f32, name="w")
    outsb = sp.tile([128, PL], f32, name="outsb")

    def v4(t):   # [128, PL, N1, N2] view
        return t[:].rearrange("p (l i j) -> p l i j", i=N1, j=N2)

    def p3(t):   # potential [128, POT] viewed [128, PL, 32]
        return t[:].rearrange("p (l x) -> p l x", x=32)

    def mid_bcast(t):
        # t: [128, (pl, j)] read as [128, pl, i(bcast), j]
        ap = t[:]
        return bass.AP(ap.tensor, ap.offset, [ap.ap[0], [N2, PL], [0, N1], [1, N2]])

    def trail_bcast(t):
        # t: [128, (pl, i)] read as [128, (pl, i), j(bcast)]
        return t[:].broadcast_to([128, POT, N2])

    def v3(t):   # [128, (pl, i), j] view of a big tile
        return t[:].rearrange("p (li j) -> p li j", j=N2)

    def strided_ij(t):
        # big tile [128, (pl, i, j)] read as [128, pl, j, i] (i innermost)
        ap = t[:]
        return bass.AP(ap.tensor, ap.offset,
                       [ap.ap[0], [N1 * N2, PL], [1, N2], [N2, N1]])

    for t in range(n_iters):
        if t == 0:
            nc.vector.tensor_reduce(out=p3(r), in_=v4(k), axis=X, op=ADD)
        else:
            nc.vector.tensor_mul(out=v[:], in0=bpre[:], in1=w[:])
            nc.vector.tensor_mul(out=v4(tmp), in0=v4(k), in1=mid_bcast(v))
            nc.vector.tensor_reduce(out=p3(r), in_=v4(tmp), axis=X, op=ADD)
        nc.vector.reciprocal(out=ri[:], in_=r[:])
        nc.vector.tensor_mul(out=u[:], in0=apre[:], in1=ri[:])
        nc.vector.tensor_mul(out=v3(tmp), in0=v3(k), in1=trail_bcast(u))
        nc.vector.tensor_reduce(out=p3(s), in_=strided_ij(tmp), axis=X, op=ADD)
        nc.vector.reciprocal(out=w[:], in_=s[:])

    # final: logits = sum_ij u * M * v,  M = K*sim*(T/32), streamed into the
    # (now dead) K tile to stay within SBUF.
    nc.vector.tensor_mul(out=v[:], in0=bpre[:], in1=w[:])
    nc.sync.dma_start(k[:], m_d.ap())
    nc.vector.tensor_mul(out=v4(tmp), in0=v4(k), in1=mid_bcast(v))
    nc.vector.tensor_mul(out=v3(k), in0=v3(tmp), in1=trail_bcast(u))
    nc.vector.tensor_reduce(out=outsb[:], in_=v4(k), axis=XY, op=ADD)
    nc.sync.dma_start(out_d.ap(), outsb[:])
    ctx.close()


_NC_CACHE = {}


def _get_program(n_iters=N_ITERS):
    if n_iters not in _NC_CACHE:
        _NC_CACHE[n_iters] = build_program(n_iters)
    return _NC_CACHE[n_iters]


def kernel(similarity_map, im_set, s_seq, im_len, s_len):
    sim = np.ascontiguousarray(np.asarray(similarity_map, dtype=np.float32))
    a = _marginals(np.asarray(im_len), N1)
    b = _marginals(np.asarray(s_len), N2)

    nc = _get_program(N_ITERS)
    in_maps = []
    for c in range(N_CORES):
        in_maps.append(pack_core(sim[c * QL:(c + 1) * QL], a[c * QL:(c + 1) * QL], b))

    from concourse.bass_utils import run_bass_kernel_spmd
    res = run_bass_kernel_spmd(nc, in_maps, core_ids=list(range(N_CORES)))
    out = np.concatenate(
        [unpack_logits(res.results[c]["out"]) for c in range(N_CORES)], axis=0
    )
    return out.astype(np.float32)
